# revision 7
# baseline (speedup 1.0000x reference)
"""ChannelSymmetry kernel for Trainium2 (8 NeuronCores, SPMD data-parallel).

Problem: X [128, 64, 8000] f32, swap_mask [128, 16] bool. For each batch b and
channel pair p (channels 2p, 2p+1; p < 16, i.e. channels 0..31), swap the two
channel rows iff swap_mask[b, p]. Channels 32..63 pass through unchanged.

Design: the permutation is runtime data, so it cannot live in compile-time DMA
access patterns. The host turns swap_mask into per-row source indices; the
device does an indirect-DMA row gather (each row = 32KB contiguous, full DMA
efficiency) from HBM into SBUF, then a regular store back to HBM. Pure DMA,
no compute engines — this is a memory-roofline problem.

Sharding: pure data parallel over the batch axis, 16 batches per core.
"""

import contextlib
import sys

import numpy as np

for _p in ("/opt/trn_rl_repo", "/opt/pypackages"):
    if _p not in sys.path:
        sys.path.append(_p)

import concourse.bass as bass
import concourse.mybir as mybir
import concourse.tile as tile
from concourse.bass_utils import run_bass_kernel_spmd

B, C, T = 128, 64, 8000
M = 8            # cores
BL = B // M      # batches per core
ROWS = BL * C    # rows per core (viewing X_shard as [ROWS, T])
P = 128          # SBUF partitions / rows per chunk


def build_bass(rows=ROWS, t=T, nbuf=3):
    """Per-core program: for each chunk of 128 rows, indirect-gather the
    permuted source rows from HBM into SBUF, then store contiguously.

    Raw bass (no Tile): walrus only allows one sync-wait per DMA
    instruction, so waits must be standalone sequencer instructions.
    gpsimd (SWDGE) issues the gathers; sync (HWDGE) issues the stores;
    two semaphores ping-pong the nbuf SBUF slots between them.
    """
    nchunk = rows // P
    nc = bass.Bass()
    x = nc.dram_tensor("x", [rows, t], mybir.dt.float32, kind="ExternalInput")
    idx = nc.dram_tensor("idx", [P, nchunk], mybir.dt.int32, kind="ExternalInput")
    y = nc.dram_tensor("y", [rows, t], mybir.dt.float32, kind="ExternalOutput")

    with contextlib.ExitStack() as ctx:
        idx_t = ctx.enter_context(
            nc.sbuf_tensor("idx_t", [P, nchunk], mybir.dt.int32)
        )
        bufs = [
            ctx.enter_context(nc.sbuf_tensor(f"buf{i}", [P, t], mybir.dt.float32))
            for i in range(nbuf)
        ]
        i_sem = ctx.enter_context(nc.semaphore(name="i_sem"))
        g_sems = [
            ctx.enter_context(nc.semaphore(name=f"g_sem{i}")) for i in range(nbuf)
        ]
        s_sems = [
            ctx.enter_context(nc.semaphore(name=f"s_sem{i}")) for i in range(nbuf)
        ]
        block = ctx.enter_context(nc.Block())

        @block.gpsimd
        def _(g):
            g.dma_start(out=idx_t[:], in_=idx[:]).then_inc(i_sem, 16)
            g.wait_ge(i_sem, 16)
            for ci in range(nchunk):
                sl, rnd = ci % nbuf, ci // nbuf
                if rnd > 0:
                    # slot free once its previous store completed
                    g.wait_ge(s_sems[sl], rnd * 16)
                g.indirect_dma_start(
                    out=bufs[sl][:],
                    out_offset=None,
                    in_=x[:],
                    in_offset=bass.IndirectOffsetOnAxis(
                        ap=idx_t[:, ci : ci + 1], axis=0
                    ),
                ).then_inc(g_sems[sl], 16)

        @block.sync
        def _(s):
            for ci in range(nchunk):
                sl, rnd = ci % nbuf, ci // nbuf
                s.wait_ge(g_sems[sl], (rnd + 1) * 16)
                s.dma_start(
                    out=y[ci * P : (ci + 1) * P, :], in_=bufs[sl][:]
                ).then_inc(s_sems[sl], 16)
            # drain: every slot's stores complete before kernel end
            for sl in range(nbuf):
                nstores = (nchunk - sl + nbuf - 1) // nbuf
                if nstores > 0:
                    s.wait_ge(s_sems[sl], nstores * 16)

    return nc


def make_in_maps(X, swap_mask):
    X = np.asarray(X, dtype=np.float32)
    swap_mask = np.asarray(swap_mask).astype(bool)
    b, c, t = X.shape

    # Source-channel permutation per batch: perm[b, ch] = channel to read.
    cidx = np.arange(c, dtype=np.int32)
    partner = np.where(cidx < 32, cidx ^ 1, cidx).astype(np.int32)
    mask_c = np.zeros((b, c), dtype=bool)
    mask_c[:, :32] = np.repeat(swap_mask, 2, axis=1)
    perm = np.where(mask_c, partner[None, :], cidx[None, :]).astype(np.int32)

    in_maps = []
    for m in range(M):
        xs = np.ascontiguousarray(X[m * BL : (m + 1) * BL].reshape(BL * c, t))
        pm = perm[m * BL : (m + 1) * BL]  # [BL, c]
        rows = (np.arange(BL, dtype=np.int32)[:, None] * c + pm).reshape(-1)
        # idx[p, chunk] = source row feeding output row chunk*P + p
        idxm = np.ascontiguousarray(rows.reshape(-1, P).T.astype(np.int32))
        in_maps.append({"x": xs, "idx": idxm})
    return in_maps


def run(X, swap_mask, **kw):
    nc = build_bass()
    if not nc.is_finalized():
        nc.finalize()
    in_maps = make_in_maps(X, swap_mask)
    res = run_bass_kernel_spmd(nc, in_maps, list(range(M)), **kw)
    out = np.concatenate(
        [r["y"].reshape(BL, C, T) for r in res.results], axis=0
    )
    return out, res


def kernel(X, swap_mask):
    out, _ = run(X, swap_mask)
    return out
